# revision 2
# baseline (speedup 1.0000x reference)
"""Trainium2 Bass kernel: softmax spatial pooling (attention pooling).

Reference computation per batch b:
    attn = softmax(probs[b].reshape(19, 16384), axis=1)
    ctx  = attn @ feats[b].reshape(512, 16384).T        # (19, 512)
    out[b] = ctx.T[..., None]                           # (512, 19, 1)

Full inputs:  feats (8, 512, 128, 128) f32, probs (8, 19, 128, 128) f32.
Sharding: pure data parallel - one batch sample per NeuronCore (8 cores).

Device-side algorithm (per core):
  View n = 16384 as (n1=128, n2=128) with n1 on SBUF partitions; the PE
  contracts over n1 (partitions) and PSUM accumulates over n2.
  Softmax is unnormalized (E = exp(p)); the 1/sum normalization is applied
  once to the small (19, C) result.

  feats path (the 32 MB that dominates): HWDGE fp32 DMA (hardware
  descriptor generation - no Q7 software descgen), then fp32->bf16 cast on
  the otherwise-idle Vector/Scalar engines, then bf16 matmuls.
    LAYOUT 'reorder': cast also transposes free dims to [n1, n2, c] so
      matmul rhs slices [:, n2, :] are contiguous (cast runs 1x strided).
    LAYOUT 'direct':  cast is contiguous [n1, c, n2] (2x mode), matmul rhs
      slices [:, :, n2] are strided.
"""

import numpy as np

import concourse.bacc as bacc
import concourse.bass as bass
import concourse.tile as tile
from concourse import mybir
from concourse.bass_utils import run_bass_kernel_spmd

B = 8          # batch == number of cores
C = 512        # feature channels
K = 19         # attention heads (probs channels)
N1 = 128       # spatial high bits -> SBUF partitions
N2 = 128       # spatial low bits  -> PSUM-accumulated matmuls

LAYOUT = "reorder"   # 'reorder' | 'direct'
CCHUNK = 128         # bf16 tile width / matmul rhs N
FCHUNK = 64          # fp32 staging piece width (channels per feats DMA)
F32_BUFS = 2
B16_BUFS = 2
CAST_ENGINES = "vs"  # round-robin: 'v' vector, 's' scalar, 'g' gpsimd

F32 = mybir.dt.float32
BF16 = mybir.dt.bfloat16


def _body(tc, pools, out, feats, probs):
    nc = tc.nc
    ffp32, ffp16, eep, smallp, csbp, pscp, pssp = pools

    # probs (K, N1*N2) -> (N1, K, N2): 512B contiguous runs per (n1, k).
    # On the scalar (Activation) HWDGE ring so it never queues behind the
    # 4MB feats transfers on the sync ring.
    probs_r = probs.rearrange("k (n1 n2) -> n1 k n2", n1=N1)
    ee = eep.tile([N1, K, N2], F32)
    nc.scalar.dma_start(out=ee[:], in_=probs_r)

    # E = exp(p) stored [n1, n2, k] so each matmul lhsT slice [:, n2, :]
    # is contiguous.
    eee = eep.tile([N1, N2, K], BF16)
    nc.scalar.activation(
        eee[:].rearrange("p n k -> p k n"), ee[:],
        mybir.ActivationFunctionType.Exp,
    )

    # partials[n1, k] = sum_n2 E[k, n1, n2]
    partials = smallp.tile([N1, K, 1], F32)
    nc.vector.reduce_sum(
        out=partials[:], in_=eee[:].rearrange("p n k -> p k n"),
        axis=mybir.AxisListType.X,
    )

    # S[k] = sum_n1 partials[n1, k] via ones-matmul; rec = 1/S per
    # partition (k on partitions)
    ones = smallp.tile([N1, 1], F32)
    nc.vector.memset(ones[:], 1.0)
    s_ps = pssp.tile([K, 1], F32)
    nc.tensor.matmul(s_ps[:], partials[:, :, 0], ones[:], start=True, stop=True)
    rec = smallp.tile([K, 1], F32)
    nc.vector.reciprocal(rec[:], s_ps[:])

    # feats (C, N1*N2) -> (N1, C, N2) view; DMA chunks of FCHUNK channels
    feats_r = feats.rearrange("c (n1 n2) -> n1 c n2", n1=N1)
    n_f_per_c = CCHUNK // FCHUNK
    cast_i = 0

    def cast(dst_view, src_view):
        nonlocal cast_i
        eng = CAST_ENGINES[cast_i % len(CAST_ENGINES)]
        cast_i += 1
        if eng == "v":
            nc.vector.tensor_copy(dst_view, src_view)
        elif eng == "s":
            nc.scalar.copy(dst_view, src_view)
        else:
            nc.gpsimd.tensor_copy(dst_view, src_view)

    for cc in range(C // CCHUNK):
        if LAYOUT == "reorder":
            ffb = ffp16.tile([N1, N2, CCHUNK], BF16)
        else:
            ffb = ffp16.tile([N1, CCHUNK, N2], BF16)
        for fc in range(n_f_per_c):
            ff32 = ffp32.tile([N1, FCHUNK, N2], F32)
            c0 = cc * CCHUNK + fc * FCHUNK
            nc.sync.dma_start(
                out=ff32[:], in_=feats_r[:, c0 : c0 + FCHUNK, :]
            )
            lo, hi = fc * FCHUNK, (fc + 1) * FCHUNK
            if LAYOUT == "reorder":
                # [p, c, n] -> [p, n, c] transposing cast
                cast(
                    ffb[:, :, lo:hi],
                    ff32[:].rearrange("p c n -> p n c"),
                )
            else:
                cast(ffb[:, lo:hi, :], ff32[:])

        c_ps = pscp.tile([K, CCHUNK], F32)
        for n2 in range(N2):
            if LAYOUT == "reorder":
                lhsT, rhs = eee[:, n2, :], ffb[:, n2, :]
            else:
                lhsT, rhs = eee[:, n2, :], ffb[:, :, n2]
            nc.tensor.matmul(
                c_ps[:], lhsT, rhs, start=(n2 == 0), stop=(n2 == N2 - 1)
            )

        # normalize: C_sb = C_ps * (1/S) per partition (= per k)
        c_sb = csbp.tile([K, CCHUNK], F32)
        nc.scalar.activation(
            c_sb[:], c_ps[:], mybir.ActivationFunctionType.Copy, scale=rec[:]
        )
        nc.scalar.dma_start(
            out=out[:, cc * CCHUNK : (cc + 1) * CCHUNK], in_=c_sb[:]
        )


_NC_CACHE = {}


def _build(reps=1):
    key = (reps, LAYOUT, CCHUNK, FCHUNK, F32_BUFS, B16_BUFS, CAST_ENGINES)
    if key in _NC_CACHE:
        return _NC_CACHE[key]
    nc = bacc.Bacc(
        "TRN2",
        target_bir_lowering=False,
        debug=False,
        num_devices=B,
    )
    feats = nc.dram_tensor("feats", [C, N1 * N2], F32, kind="ExternalInput").ap()
    probs = nc.dram_tensor("probs", [K, N1 * N2], F32, kind="ExternalInput").ap()
    out = nc.dram_tensor("out", [K, C], F32, kind="ExternalOutput").ap()
    with tile.TileContext(nc) as tc:
        with (
            tc.tile_pool(name="ff32", bufs=F32_BUFS) as ffp32,
            tc.tile_pool(name="ff16", bufs=B16_BUFS) as ffp16,
            tc.tile_pool(name="ee", bufs=2) as eep,
            tc.tile_pool(name="small", bufs=2) as smallp,
            tc.tile_pool(name="csb", bufs=2) as csbp,
            tc.tile_pool(name="psc", bufs=2, space="PSUM") as pscp,
            tc.tile_pool(name="pss", bufs=2, space="PSUM") as pssp,
        ):
            pools = (ffp32, ffp16, eep, smallp, csbp, pscp, pssp)
            for _ in range(reps):
                _body(tc, pools, out, feats, probs)
    nc.compile()
    _NC_CACHE[key] = nc
    return nc


def kernel(feats: np.ndarray, probs: np.ndarray) -> np.ndarray:
    assert feats.shape == (B, C, N1, N2) and probs.shape == (B, K, N1, N2)
    nc = _build()
    in_maps = [
        {
            "feats": np.ascontiguousarray(feats[b]).reshape(C, N1 * N2),
            "probs": np.ascontiguousarray(probs[b]).reshape(K, N1 * N2),
        }
        for b in range(B)
    ]
    res = run_bass_kernel_spmd(nc, in_maps, core_ids=list(range(B)))
    out = np.stack([res.results[b]["out"] for b in range(B)])
    out = out.transpose(0, 2, 1)  # (B, K, C) -> (B, C, K)
    return np.ascontiguousarray(out)[..., None].astype(np.float32)


if __name__ == "__main__":
    rng = np.random.default_rng(0)
    f = rng.standard_normal((B, C, N1, N2), dtype=np.float32)
    p = rng.standard_normal((B, K, N1, N2), dtype=np.float32)
    o = kernel(f, p)
    print("out", o.shape, o.dtype)
